# revision 22
# baseline (speedup 1.0000x reference)
"""Trainium2 Bass kernel for a 4-term video/query contrastive loss.

Strategy (v10): data-parallel over batch B=64 across 8 cores (8 videos/core).
Device computes only the big cross-contrast term: per chunk of 128 proposals,
66 weight rows (64 queries + the chunk's own video's 2 top-1 features) are
scored against the proposals, exp'd at 1/temperature, and mask-reduced per
(video, {valid, iou>0.5}).

  - host pre-normalizes everything and pads each video's 2080 upper-tri
    proposals to 2176 = 17*128, so every chunk belongs to exactly one video
  - V ships fp8e4 chunk-major [128, 1056 + 136*256] (per-video W variants
    packed in front, both C-halves interleaved per chunk); each slab is ONE
    dma_start with a single contiguous segment per partition
  - slabs alternate between the two hardware DGE rings (sync + scalar) in
    consumption order; together they sustain ~410 GB/s
  - scores transposed S^T[p, r]: weights = fp8 v-chunks, stream = the
    chunk's W variant (N=66), two C-half matmuls accumulate in PSUM;
    12 chunks per 2-bank PSUM group, one Exp ([128, 2, 396]) -> bf16
  - one mask matmul per 6-chunk subgroup: lhsT = [128, 96] block mask
    (16 absolute (video, {valid,pos}) cols per chunk), rhs = et
    [128, 396], PSUM-accumulated into two [96, 396] halves; the first
    half's copy+store overlaps the steady phase
  - host sums the diagonal [16, 66] blocks and computes the tiny loss
    terms (top-k gather, pos-pair cosines, logs) in float64
"""

import numpy as np
import ml_dtypes

import concourse.bacc as bacc
import concourse.bass as bass
import concourse.tile as tile
from concourse import mybir
from concourse import bass_utils

f32 = mybir.dt.float32
bf16 = mybir.dt.bfloat16
AFT = mybir.ActivationFunctionType
BF = ml_dtypes.bfloat16

DT = mybir.dt.float8e4
DTNP = ml_dtypes.float8_e4m3

B, C, D = 64, 256, 64
T = 128
P = 2080                    # real upper-tri positions per video
PPAD = 2176                 # padded to 17 * 128
NCORES = 8
VB = B // NCORES            # videos per core: 8
NPT = 2                     # sentences per video
NPROP = VB * PPAD           # 17408 padded proposals per core
PCH = 128                   # proposals per chunk (partition dim of S^T)
NCH = NPROP // PCH          # 136 chunks (17 per video)
CPV = PPAD // PCH           # 17 chunks per video
G = 6                       # chunks per mask subgroup (6*66 = 396 f32)
NG = (NCH + G - 1) // G     # 23 subgroups (22 full + 1 of 4)
NW = B + NPT                # 66 score rows: 64 queries + own video's 2 topk
MC = 2 * VB                 # 16 mask cols: (valid, iou>0.5) per local video
GM = G * MC                 # 96 block-mask cols per subgroup
WCOLS = 2 * NW              # 132 W cols per video variant (2 C-halves)
WOFF = VB * WCOLS           # 1056 flat cols of W variants in front of d_v
VCOLS = WOFF + NCH * 2 * PCH
TAU = 10.0
NEG_IOU = 0.5

SG = 2 * G                  # 12 chunks per super-group (2 PSUM banks)
NSG = (NCH + SG - 1) // SG  # 12 super-groups (11 full + 1 of 4)
BANKF = 512                 # f32 elements per PSUM bank
NG_A = 12                   # subgroups accumulated into the first rsum half

# v-load slabs (in chunks): (start, n, ring)  ring 0=sync, 1=scalar.
# Slab 0 also carries the W variants. Ranges alternate between the two
# FIFO rings in consumption order so the SDMA engines always work on the
# next-needed data.
VSLABS = [(0, 4, 0), (4, 12, 1), (16, 12, 0), (28, 12, 1), (40, 12, 0),
          (52, 12, 1), (64, 12, 0), (76, 12, 1), (88, 12, 0), (100, 12, 1),
          (112, 12, 0), (124, 12, 1)]
MASK_LAG = 2                # super-groups of score-MM lead over mask MMs


def _flat(c):
    return WOFF + c * 2 * PCH


def _build_module():
    nc = bacc.Bacc("TRN2", target_bir_lowering=False, debug=False)

    d_v = nc.dram_tensor("v8", (PCH, VCOLS), DT, kind="ExternalInput")
    d_m = nc.dram_tensor("msk", (PCH, NG * GM), bf16, kind="ExternalInput")
    d_or = nc.dram_tensor("o_r", (2, GM, G * NW), bf16, kind="ExternalOutput")

    with tile.TileContext(nc) as tc:
        with (
            tc.tile_pool(name="consts", bufs=1) as cp,
            tc.tile_pool(name="ets", bufs=4) as ep,
            tc.tile_pool(name="outs", bufs=1) as op_,
            tc.tile_pool(name="ps", bufs=3, space="PSUM") as ps,
            tc.tile_pool(name="pra", bufs=1, space="PSUM") as pra,
            tc.tile_pool(name="prb", bufs=1, space="PSUM") as prb,
        ):
            mt = cp.tile([PCH, NG * GM], bf16, tag="mt")
            slab_of = {}
            vts = []
            n_scalar = 0
            for si, (c0, nch, ring) in enumerate(VSLABS):
                a = _flat(c0) if si else 0
                b = _flat(c0 + nch)
                vtile = cp.tile([PCH, b - a], DT, tag=f"v{si}", name=f"v{si}")
                if ring == 0:
                    nc.sync.dma_start(vtile, d_v[:, a:b])
                else:
                    n_scalar += 1
                    nc.scalar.dma_start(vtile, d_v[:, a:b])
                    if n_scalar == 1:
                        nc.scalar.dma_start(mt, d_m[:])
                vts.append((a, vtile))
                for c in range(c0, c0 + nch):
                    slab_of[c] = si
            wt = vts[0][1]      # W variants live at the front of slab 0

            rsA = pra.tile([GM, G * NW], f32, tag="rsA")
            rsB = prb.tile([GM, G * NW], f32, tag="rsB")
            ets = []            # et tile per super-group

            def mask_mm(g):
                sg, jj = divmod(g, 2)
                rs = rsA if g < NG_A else rsB
                nc.tensor.matmul(rs, mt[:, g * GM:(g + 1) * GM],
                                 ets[sg][:, jj, :],
                                 start=(g in (0, NG_A)),
                                 stop=(g in (NG_A - 1, NG - 1)))

            rs_sbA = op_.tile([GM, G * NW], bf16, tag="rsbA")
            rs_sbB = op_.tile([GM, G * NW], bf16, tag="rsbB")

            for sg in range(NSG):
                cg = min(SG, NCH - sg * SG)
                st = ps.tile([128, 2, BANKF], f32, tag="st")
                for j in range(cg):
                    c = sg * SG + j
                    jj, m = divmod(j, G)
                    a, vtile = vts[slab_of[c]]
                    o = _flat(c) - a
                    v = c // CPV
                    ds = st[:, jj, m * NW:(m + 1) * NW]
                    nc.tensor.matmul(ds, vtile[:, o:o + PCH],
                                     wt[:, v * WCOLS:v * WCOLS + NW],
                                     start=(m == 0), stop=False)
                    nc.tensor.matmul(ds, vtile[:, o + PCH:o + 2 * PCH],
                                     wt[:, v * WCOLS + NW:v * WCOLS + 2 * NW],
                                     start=False, stop=(m == G - 1 or j == cg - 1))
                et = ep.tile([128, 2, G * NW], bf16, tag="et")
                if cg < SG:
                    nc.vector.memset(et, 0.0)
                    r = cg if cg < G else G
                    nc.scalar.activation(et[:, 0, 0:r * NW], st[:, 0, 0:r * NW],
                                         AFT.Exp, scale=TAU)
                    if cg > G:
                        r = cg - G
                        nc.scalar.activation(et[:, 1, 0:r * NW],
                                             st[:, 1, 0:r * NW],
                                             AFT.Exp, scale=TAU)
                else:
                    nc.scalar.activation(et[:, :, :], st[:, :, 0:G * NW],
                                         AFT.Exp, scale=TAU)
                ets.append(et)
                if sg >= MASK_LAG:
                    g = 2 * (sg - MASK_LAG)
                    mask_mm(g)
                    mask_mm(g + 1)
                    if g + 1 == NG_A - 1:   # first half complete: ship it now
                        nc.vector.tensor_copy(rs_sbA, rsA)
                        nc.sync.dma_start(d_or[0], rs_sbA)
            for g in range(2 * (NSG - MASK_LAG), NG):
                mask_mm(g)

            nc.vector.tensor_copy(rs_sbB, rsB)
            nc.sync.dma_start(d_or[1], rs_sbB)

    nc.compile()
    return nc


_MODULE = None


def _get_module():
    global _MODULE
    if _MODULE is None:
        _MODULE = _build_module()
    return _MODULE


def kernel(video_feats, query_feats, sents_feats, iou2d, iou2ds, num_targets):
    video_feats = np.ascontiguousarray(np.asarray(video_feats, np.float32))
    query_feats = np.asarray(query_feats, np.float32)
    sents_feats = np.asarray(sents_feats, np.float32)
    iou2d = np.asarray(iou2d, np.float32)
    iou2ds = np.asarray(iou2ds, np.float32)
    nt = np.asarray(num_targets)
    assert video_feats.shape == (B, C, D, D) and sents_feats.shape == (T, C)
    assert (nt == NPT).all(), "kernel assumes uniform num_targets == 2"

    rows, cols = np.triu_indices(D)
    tri = rows * D + cols

    vf = video_feats.reshape(B, C, D * D)[:, :, tri]           # (64, 256, 2080)
    nrm = np.sqrt(np.einsum('bcp,bcp->bp', vf, vf))
    vhat = vf / np.maximum(nrm, 1e-12)[:, None, :]

    qn = query_feats / np.maximum(
        np.linalg.norm(query_feats, axis=1, keepdims=True), 1e-12)
    sn = sents_feats / np.maximum(
        np.linalg.norm(sents_feats, axis=1, keepdims=True), 1e-12)

    iouf = iou2ds.reshape(T, D * D)[:, tri]
    pstar = iouf.argmax(1)                                     # top-1 per sentence
    scatter = np.repeat(np.arange(B), NPT)
    tvn = vhat[scatter, :, pstar]                              # (128, 256) normalized
    iou_tri = iou2d.reshape(B, D * D)[:, tri]
    posm_all = iou_tri > NEG_IOU

    pidx = np.tile(np.arange(PPAD), VB)                        # pos within video
    vidx = np.repeat(np.arange(VB), PPAD)
    real = pidx < P
    ar = np.arange(NPROP)
    in_maps = []
    for k in range(NCORES):
        g0 = VB * k
        # padded, chunk-major, C-half interleaved V with W variants in front
        vp8 = np.zeros((PCH, VCOLS), DTNP)
        vpad = np.zeros((2, PCH, VB, PPAD), np.float32)
        vpad[:, :, :, :P] = vhat[g0:g0 + VB].transpose(1, 0, 2).reshape(
            2, PCH, VB, P)
        vp8[:, WOFF:] = vpad.reshape(2, PCH, NCH, PCH).transpose(
            1, 2, 0, 3).reshape(PCH, NCH * 2 * PCH).astype(DTNP)
        for v in range(VB):
            wv = np.concatenate([qn, tvn[2 * (g0 + v):2 * (g0 + v) + 2]], 0)
            wv = wv.T.reshape(2, PCH, NW)                      # (half, 128, 66)
            vp8[:, v * WCOLS:v * WCOLS + NW] = wv[0].astype(DTNP)
            vp8[:, v * WCOLS + NW:(v + 1) * WCOLS] = wv[1].astype(DTNP)

        m = np.zeros((NG * G * PCH, MC), np.float32)
        pos = np.zeros(NPROP, bool)
        pos[real] = posm_all[g0:g0 + VB].reshape(-1)
        m[ar, 2 * vidx] = real
        m[ar, 2 * vidx + 1] = pos
        mh = m.reshape(NG, G, PCH, MC).transpose(2, 0, 1, 3).reshape(
            PCH, NG * GM).astype(BF)
        in_maps.append({
            "v8": vp8,
            "msk": np.ascontiguousarray(mh),
        })

    nc = _get_module()
    res = bass_utils.run_bass_kernel_spmd(nc, in_maps, core_ids=list(range(NCORES)))
    kernel._last = res
    outs = res.results

    # ---- host finalization (tiny, float64) ----
    E = np.float64
    valid = np.zeros((NCORES, VB, NW))
    posv = np.zeros((NCORES, VB, NW))
    for k in range(NCORES):
        rs = outs[k]["o_r"].astype(E).sum(0)                   # (96, 396)
        acc = np.zeros((MC, NW))
        for j in range(G):
            acc += rs[MC * j:MC * (j + 1), NW * j:NW * (j + 1)]
        valid[k] = acc[0::2, :]
        posv[k] = acc[1::2, :]

    tvn64, qn64, sn64 = tvn.astype(E), qn.astype(E), sn.astype(E)
    negq = valid[:, :, :B].sum(axis=(0, 1))                    # (64,)
    for b in range(B):
        negq[b] -= posv[b // VB, b % VB, b]

    pos_t = (tvn64 * qn64[scatter]).sum(1)                     # (128,)
    E1 = np.exp(TAU * qn64 @ tvn64.T)                          # (64, 128)
    asum = E1.sum(0)
    t1 = -(TAU * pos_t - np.log(asum))
    t2 = -(TAU * pos_t - np.log(np.exp(TAU * pos_t) + negq[scatter]))

    a3 = tvn64 @ tvn64.T
    t3 = []
    for g in range(B):
        k, v = g // VB, g % VB
        for i in (NPT * g, NPT * g + 1):
            r = B + (i - NPT * g)                              # col 64 or 65
            neg_i = valid[k, v, r] - posv[k, v, r]
            for j in (NPT * g, NPT * g + 1):
                pd = a3[i, j]
                t3.append(-(TAU * pd - np.log(np.exp(TAU * pd) + neg_i)))

    pos4 = (sn64 * qn64[scatter]).sum(1)
    E4 = np.exp(TAU * qn64 @ sn64.T)                           # (64, 128)
    mask4 = (scatter[None, :] != np.arange(B)[:, None])
    negsum4 = (E4 * mask4).sum(1)
    t4 = -(TAU * pos4 - np.log(np.exp(TAU * pos4) + negsum4[scatter]))

    return np.stack([t1.mean(), t2.mean(), np.mean(t3), t4.mean()]).astype(np.float32)


# revision 23
# speedup vs baseline: 1.0681x; 1.0681x over previous
"""Trainium2 Bass kernel for a 4-term video/query contrastive loss.

Strategy (v10): data-parallel over batch B=64 across 8 cores (8 videos/core).
Device computes only the big cross-contrast term: per chunk of 128 proposals,
66 weight rows (64 queries + the chunk's own video's 2 top-1 features) are
scored against the proposals, exp'd at 1/temperature, and mask-reduced per
(video, {valid, iou>0.5}).

  - host pre-normalizes everything and pads each video's 2080 upper-tri
    proposals to 2176 = 17*128, so every chunk belongs to exactly one video
  - V ships fp8e4 chunk-major [128, 1056 + 136*256] (per-video W variants
    packed in front, both C-halves interleaved per chunk); each slab is ONE
    dma_start with a single contiguous segment per partition
  - slabs alternate between the two hardware DGE rings (sync + scalar) in
    consumption order; together they sustain ~410 GB/s
  - scores transposed S^T[p, r]: weights = fp8 v-chunks, stream = the
    chunk's W variant (N=66), two C-half matmuls accumulate in PSUM;
    12 chunks per 2-bank PSUM group, one Exp ([128, 2, 396]) -> bf16
  - one mask matmul per 6-chunk subgroup: lhsT = [128, 96] block mask
    (16 absolute (video, {valid,pos}) cols per chunk), rhs = et
    [128, 396], PSUM-accumulated into two [96, 396] halves; the first
    half's copy+store overlaps the steady phase
  - host sums the diagonal [16, 66] blocks and computes the tiny loss
    terms (top-k gather, pos-pair cosines, logs) in float64
"""

import numpy as np
import ml_dtypes

import concourse.bacc as bacc
import concourse.bass as bass
import concourse.tile as tile
from concourse import mybir
from concourse import bass_utils

f32 = mybir.dt.float32
bf16 = mybir.dt.bfloat16
AFT = mybir.ActivationFunctionType
BF = ml_dtypes.bfloat16

DT = mybir.dt.float8e4
DTNP = ml_dtypes.float8_e4m3

B, C, D = 64, 256, 64
T = 128
P = 2080                    # real upper-tri positions per video
PPAD = 2176                 # padded to 17 * 128
NCORES = 8
VB = B // NCORES            # videos per core: 8
NPT = 2                     # sentences per video
NPROP = VB * PPAD           # 17408 padded proposals per core
PCH = 128                   # proposals per chunk (partition dim of S^T)
NCH = NPROP // PCH          # 136 chunks (17 per video)
CPV = PPAD // PCH           # 17 chunks per video
G = 6                       # chunks per mask subgroup (6*66 = 396 f32)
NG = (NCH + G - 1) // G     # 23 subgroups (22 full + 1 of 4)
NW = B + NPT                # 66 score rows: 64 queries + own video's 2 topk
MC = 2 * VB                 # 16 mask cols: (valid, iou>0.5) per local video
GM = G * MC                 # 96 block-mask cols per subgroup
WCOLS = 2 * NW              # 132 W cols per video variant (2 C-halves)
WOFF = VB * WCOLS           # 1056 flat cols of W variants in front of d_v
VCOLS = WOFF + NCH * 2 * PCH
TAU = 10.0
NEG_IOU = 0.5

SG = 2 * G                  # 12 chunks per super-group (2 PSUM banks)
NSG = (NCH + SG - 1) // SG  # 12 super-groups (11 full + 1 of 4)
BANKF = 512                 # f32 elements per PSUM bank
NG_A = 12                   # subgroups accumulated into the first rsum half

# Transfer schedule: the stream is supply-limited, so every transfer sits
# on one of the two FIFO hardware DGE rings (0=sync, 1=scalar) in NEED
# order. ('v', start, n) = v-chunk slab, ('m', g0, g1) = mask groups.
# Slab 0 also carries the W variants. Mask groups >= 18 arrive last, so
# their matmuls are deferred past the final score matmul.
SCHED = [
    (0, ('v', 0, 4)), (1, ('v', 4, 16)), (0, ('v', 16, 40)),
    (1, ('v', 40, 64)), (0, ('m', 0, 6)), (0, ('v', 64, 88)),
    (1, ('m', 6, 12)), (1, ('v', 88, 112)), (0, ('m', 12, 18)),
    (0, ('v', 112, 136)), (1, ('m', 18, 23)),
]
MDEFER = 18                 # mask groups >= this run after the last score MM
MASK_LAG = 2                # super-groups of score-MM lead over mask MMs


def _flat(c):
    return WOFF + c * 2 * PCH


def _build_module():
    nc = bacc.Bacc("TRN2", target_bir_lowering=False, debug=False)

    d_v = nc.dram_tensor("v8", (PCH, VCOLS), DT, kind="ExternalInput")
    d_m = nc.dram_tensor("msk", (PCH, NG * GM), bf16, kind="ExternalInput")
    d_or = nc.dram_tensor("o_r", (2, GM, G * NW), bf16, kind="ExternalOutput")

    with tile.TileContext(nc) as tc:
        with (
            tc.tile_pool(name="consts", bufs=1) as cp,
            tc.tile_pool(name="ets", bufs=4) as ep,
            tc.tile_pool(name="outs", bufs=1) as op_,
            tc.tile_pool(name="ps", bufs=3, space="PSUM") as ps,
            tc.tile_pool(name="pra", bufs=1, space="PSUM") as pra,
            tc.tile_pool(name="prb", bufs=1, space="PSUM") as prb,
        ):
            mt = cp.tile([PCH, NG * GM], bf16, tag="mt")
            slab_of = {}
            vts = []
            for ring, item in SCHED:
                eng = nc.sync if ring == 0 else nc.scalar
                if item[0] == 'v':
                    _, c0, c1 = item
                    a = _flat(c0) if c0 else 0
                    b = _flat(c1)
                    vtile = cp.tile([PCH, b - a], DT, tag=f"v{c0}",
                                    name=f"v{c0}")
                    eng.dma_start(vtile, d_v[:, a:b])
                    vts.append((a, vtile))
                    for c in range(c0, c1):
                        slab_of[c] = len(vts) - 1
                else:
                    _, g0, g1 = item
                    eng.dma_start(mt[:, g0 * GM:g1 * GM],
                                  d_m[:, g0 * GM:g1 * GM])
            wt = vts[0][1]      # W variants live at the front of slab 0

            rsA = pra.tile([GM, G * NW], f32, tag="rsA")
            rsB = prb.tile([GM, G * NW], f32, tag="rsB")
            ets = []            # et tile per super-group

            def mask_mm(g):
                sg, jj = divmod(g, 2)
                rs = rsA if g < NG_A else rsB
                nc.tensor.matmul(rs, mt[:, g * GM:(g + 1) * GM],
                                 ets[sg][:, jj, :],
                                 start=(g in (0, NG_A)),
                                 stop=(g in (NG_A - 1, NG - 1)))

            rs_sbA = op_.tile([GM, G * NW], bf16, tag="rsbA")
            rs_sbB = op_.tile([GM, G * NW], bf16, tag="rsbB")

            for sg in range(NSG):
                cg = min(SG, NCH - sg * SG)
                st = ps.tile([128, 2, BANKF], f32, tag="st")
                for j in range(cg):
                    c = sg * SG + j
                    jj, m = divmod(j, G)
                    a, vtile = vts[slab_of[c]]
                    o = _flat(c) - a
                    v = c // CPV
                    ds = st[:, jj, m * NW:(m + 1) * NW]
                    nc.tensor.matmul(ds, vtile[:, o:o + PCH],
                                     wt[:, v * WCOLS:v * WCOLS + NW],
                                     start=(m == 0), stop=False)
                    nc.tensor.matmul(ds, vtile[:, o + PCH:o + 2 * PCH],
                                     wt[:, v * WCOLS + NW:v * WCOLS + 2 * NW],
                                     start=False, stop=(m == G - 1 or j == cg - 1))
                et = ep.tile([128, 2, G * NW], bf16, tag="et")
                if cg < SG:
                    nc.vector.memset(et, 0.0)
                    r = cg if cg < G else G
                    nc.scalar.activation(et[:, 0, 0:r * NW], st[:, 0, 0:r * NW],
                                         AFT.Exp, scale=TAU)
                    if cg > G:
                        r = cg - G
                        nc.scalar.activation(et[:, 1, 0:r * NW],
                                             st[:, 1, 0:r * NW],
                                             AFT.Exp, scale=TAU)
                else:
                    nc.scalar.activation(et[:, :, :], st[:, :, 0:G * NW],
                                         AFT.Exp, scale=TAU)
                ets.append(et)
                if sg >= MASK_LAG:
                    g = 2 * (sg - MASK_LAG)
                    for gg in (g, g + 1):
                        if gg < MDEFER:
                            mask_mm(gg)
                    if g + 1 == NG_A - 1:   # first half complete: ship it now
                        nc.vector.tensor_copy(rs_sbA, rsA)
                        nc.sync.dma_start(d_or[0], rs_sbA)
            for g in range(MDEFER, NG):
                mask_mm(g)

            nc.vector.tensor_copy(rs_sbB, rsB)
            nc.sync.dma_start(d_or[1], rs_sbB)

    nc.compile()
    return nc


_MODULE = None


def _get_module():
    global _MODULE
    if _MODULE is None:
        _MODULE = _build_module()
    return _MODULE


def kernel(video_feats, query_feats, sents_feats, iou2d, iou2ds, num_targets):
    video_feats = np.ascontiguousarray(np.asarray(video_feats, np.float32))
    query_feats = np.asarray(query_feats, np.float32)
    sents_feats = np.asarray(sents_feats, np.float32)
    iou2d = np.asarray(iou2d, np.float32)
    iou2ds = np.asarray(iou2ds, np.float32)
    nt = np.asarray(num_targets)
    assert video_feats.shape == (B, C, D, D) and sents_feats.shape == (T, C)
    assert (nt == NPT).all(), "kernel assumes uniform num_targets == 2"

    rows, cols = np.triu_indices(D)
    tri = rows * D + cols

    vf = video_feats.reshape(B, C, D * D)[:, :, tri]           # (64, 256, 2080)
    nrm = np.sqrt(np.einsum('bcp,bcp->bp', vf, vf))
    vhat = vf / np.maximum(nrm, 1e-12)[:, None, :]

    qn = query_feats / np.maximum(
        np.linalg.norm(query_feats, axis=1, keepdims=True), 1e-12)
    sn = sents_feats / np.maximum(
        np.linalg.norm(sents_feats, axis=1, keepdims=True), 1e-12)

    iouf = iou2ds.reshape(T, D * D)[:, tri]
    pstar = iouf.argmax(1)                                     # top-1 per sentence
    scatter = np.repeat(np.arange(B), NPT)
    tvn = vhat[scatter, :, pstar]                              # (128, 256) normalized
    iou_tri = iou2d.reshape(B, D * D)[:, tri]
    posm_all = iou_tri > NEG_IOU

    pidx = np.tile(np.arange(PPAD), VB)                        # pos within video
    vidx = np.repeat(np.arange(VB), PPAD)
    real = pidx < P
    ar = np.arange(NPROP)
    in_maps = []
    for k in range(NCORES):
        g0 = VB * k
        # padded, chunk-major, C-half interleaved V with W variants in front
        vp8 = np.zeros((PCH, VCOLS), DTNP)
        vpad = np.zeros((2, PCH, VB, PPAD), np.float32)
        vpad[:, :, :, :P] = vhat[g0:g0 + VB].transpose(1, 0, 2).reshape(
            2, PCH, VB, P)
        vp8[:, WOFF:] = vpad.reshape(2, PCH, NCH, PCH).transpose(
            1, 2, 0, 3).reshape(PCH, NCH * 2 * PCH).astype(DTNP)
        for v in range(VB):
            wv = np.concatenate([qn, tvn[2 * (g0 + v):2 * (g0 + v) + 2]], 0)
            wv = wv.T.reshape(2, PCH, NW)                      # (half, 128, 66)
            vp8[:, v * WCOLS:v * WCOLS + NW] = wv[0].astype(DTNP)
            vp8[:, v * WCOLS + NW:(v + 1) * WCOLS] = wv[1].astype(DTNP)

        m = np.zeros((NG * G * PCH, MC), np.float32)
        pos = np.zeros(NPROP, bool)
        pos[real] = posm_all[g0:g0 + VB].reshape(-1)
        m[ar, 2 * vidx] = real
        m[ar, 2 * vidx + 1] = pos
        mh = m.reshape(NG, G, PCH, MC).transpose(2, 0, 1, 3).reshape(
            PCH, NG * GM).astype(BF)
        in_maps.append({
            "v8": vp8,
            "msk": np.ascontiguousarray(mh),
        })

    nc = _get_module()
    res = bass_utils.run_bass_kernel_spmd(nc, in_maps, core_ids=list(range(NCORES)))
    kernel._last = res
    outs = res.results

    # ---- host finalization (tiny, float64) ----
    E = np.float64
    valid = np.zeros((NCORES, VB, NW))
    posv = np.zeros((NCORES, VB, NW))
    for k in range(NCORES):
        rs = outs[k]["o_r"].astype(E).sum(0)                   # (96, 396)
        acc = np.zeros((MC, NW))
        for j in range(G):
            acc += rs[MC * j:MC * (j + 1), NW * j:NW * (j + 1)]
        valid[k] = acc[0::2, :]
        posv[k] = acc[1::2, :]

    tvn64, qn64, sn64 = tvn.astype(E), qn.astype(E), sn.astype(E)
    negq = valid[:, :, :B].sum(axis=(0, 1))                    # (64,)
    for b in range(B):
        negq[b] -= posv[b // VB, b % VB, b]

    pos_t = (tvn64 * qn64[scatter]).sum(1)                     # (128,)
    E1 = np.exp(TAU * qn64 @ tvn64.T)                          # (64, 128)
    asum = E1.sum(0)
    t1 = -(TAU * pos_t - np.log(asum))
    t2 = -(TAU * pos_t - np.log(np.exp(TAU * pos_t) + negq[scatter]))

    a3 = tvn64 @ tvn64.T
    t3 = []
    for g in range(B):
        k, v = g // VB, g % VB
        for i in (NPT * g, NPT * g + 1):
            r = B + (i - NPT * g)                              # col 64 or 65
            neg_i = valid[k, v, r] - posv[k, v, r]
            for j in (NPT * g, NPT * g + 1):
                pd = a3[i, j]
                t3.append(-(TAU * pd - np.log(np.exp(TAU * pd) + neg_i)))

    pos4 = (sn64 * qn64[scatter]).sum(1)
    E4 = np.exp(TAU * qn64 @ sn64.T)                           # (64, 128)
    mask4 = (scatter[None, :] != np.arange(B)[:, None])
    negsum4 = (E4 * mask4).sum(1)
    t4 = -(TAU * pos4 - np.log(np.exp(TAU * pos4) + negsum4[scatter]))

    return np.stack([t1.mean(), t2.mean(), np.mean(t3), t4.mean()]).astype(np.float32)
